# revision 6
# baseline (speedup 1.0000x reference)
"""Trainium2 Bass kernel for nn_LstmRNN: 8-core DATA-parallel LSTM, fp8 W_hh.

Strategy: batch 64 split 8 ways (8 per core). Each core runs the FULL
H=1024 recurrence for its batch slice - zero inter-core communication.
This replaces the old tensor-parallel design whose per-step 8-core
h all-gather cost ~12.9us of every 15.4us step (measured: ncfw
collectives have a hard ~7.8us/op throughput floor regardless of size,
and the SBUF->SBUF remote-DMA path is both unstable for multi-dest
broadcasts and ~50x below nominal bandwidth for singletons).

The DP per-step cost is LDWEIGHTS-bound: every step streams all of
W_hh through the PE as stationary operands. Storing W_hh in fp8-e4m3
(scaled x128, compensated by h_scaled = h/128 and W_o x128) halves the
FWL weight-load time vs bf16. Per-gate PSUM banks are split into lo/hi
hidden-chunk halves (8 banks, bufs=1) and the step is software-
pipelined: the next step's k=0..3 matmuls need only h_lo, which is
ready before this step's matmuls finish, so the PE never stalls on the
EW tail (sem-prop ~0.9us + sigmoid + mult).

Measured on trn2 (neuron-profile, traced): 1.24 ms total vs 2.14 ms
baseline. Phase 2 is ~9.0us/step = 264 LDW+MM pairs x ~34ns; the pace
is set by the HAM clock gate, which keeps the PE at K=4/8 (1.2 GHz)
for the whole recurrence - the N=8 streams never look "busy" enough
to unthrottle it (phase 1, N=512, runs at K=8/8). Warming attempts
with interleaved dummy N=512 matmuls did not flip it.

Numerics: fp8 W_hh + bf16 h -> rel err 3.3e-3 vs fp32 ref (gate 2e-2).

Pace model (settled by experiment): steady-state 34.2ns per
LDWEIGHTS+MATMUL pair IS the array's true throughput at K=4/8
(26.7ns fp8-FWL LDW + ~7.5ns unhidden MM/bookkeeping). Traces show
~34 pairs at 26-27ns right after any PE dependency stall, but every
attempt to exploit this lost or broke even (natural 273ns stall
refunds ~255ns; a 1382ns stall still refunds <=255ns; inserted
satisfied-wait bubbles refunded nothing, +85us): the "fast window"
is the issue queue REFILLING after a drain - timestamp compression,
not faster execution. No bubble scheme can profit. The only real
phase-2 lever is HAM K=8/8 itself (~2x LDW rate), which N=8 streams
cannot earn: un-throttle requires sustained high streaming duty
(phase 1's N=512 runs at K=8/8; dummy-matmul warming failed).

Layouts (per core, host-prepped):
  gates col-permuted to blocks [i, f, o, g] (orig i,f,g,o).
  xg^T   SBUF [128, 32 chunk, T*8] bf16  (chunk m = gate dims 128m..128m+127)
  whh_q  SBUF [128, 8 k, 4096] fp8       (W_hh[:, cols] * 128)
  h^T    SBUF 2x [128, 4 k, 8] bf16      (= h / 128, lo/hi halves)
  gates^T PSUM per (gate, half): [128, 4 chunk, 8 batch] fp32
"""

import sys

for _p in ("/opt/trn_rl_repo",):
    if _p not in sys.path:
        sys.path.insert(0, _p)

import numpy as np
import ml_dtypes

import concourse.bass as bass
import concourse.mybir as mybir
import concourse.tile as tile
from concourse import bacc
from concourse import bass_utils

BF16 = ml_dtypes.bfloat16
FP8 = ml_dtypes.float8_e4m3

B, T, I, H, O = 64, 128, 512, 1024, 512
NCORES = 8
BS = B // NCORES           # 8 batch per core
NM = 4 * H // 128          # 32 gate chunks
WS = 128.0                 # fp8 weight scale (h carries 1/WS)

F32 = mybir.dt.float32
BF = mybir.dt.bfloat16
F8 = mybir.dt.float8e4
AF = mybir.ActivationFunctionType
ALU = mybir.AluOpType

# col-block index of each gate after the [i, f, o, g] permutation
BLK = {"i": 0, "f": 1, "o": 2, "g": 3}
ORDER = ["g", "i", "f", "o"]


def build_program(t_steps: int = T, wdtype=F8):
    nc = bacc.Bacc(
        "TRN2",
        target_bir_lowering=False,
        debug=False,
        num_devices=NCORES,
    )

    xs_t = nc.dram_tensor("xs_t", [I, T * BS], BF, kind="ExternalInput")
    wih = nc.dram_tensor("wih", [I, 4 * H], BF, kind="ExternalInput")
    whh = nc.dram_tensor("whh", [H, 4 * H], wdtype, kind="ExternalInput")
    bias = nc.dram_tensor("bias", [128, NM], F32, kind="ExternalInput")
    wo = nc.dram_tensor("wo", [H, O], BF, kind="ExternalInput")
    bo = nc.dram_tensor("bo", [128, O // 128], F32, kind="ExternalInput")
    ident = nc.dram_tensor("ident", [128, 128], BF, kind="ExternalInput")
    out_t = nc.dram_tensor("out_t", [O, BS], F32, kind="ExternalOutput")

    with tile.TileContext(nc) as tc:
        with (
            tc.tile_pool(name="consts", bufs=1) as consts,
            tc.tile_pool(name="xg", bufs=1) as xgp,
            tc.tile_pool(name="xsp", bufs=2) as xsp,
            tc.tile_pool(name="psum", bufs=1, space="PSUM") as psp,
            tc.tile_pool(name="ew", bufs=2) as ew,
        ):
            # ---- constants into SBUF ----
            # Phase-1-critical loads (xs, bias, then wih in column blocks)
            # go on the two HWDGE queues; bulky whh/wo (needed only at
            # phase 2/3) go on the otherwise-idle gpsimd SWDGE queue.
            xs_sb = consts.tile([128, 4, T * BS], BF)
            nc.sync.dma_start(xs_sb[:], xs_t.rearrange("(k p) c -> p k c", p=128))
            bias_sb = consts.tile([128, NM], F32)
            nc.scalar.dma_start(bias_sb[:], bias[:, :])
            id_sb = consts.tile([128, 128], BF)
            nc.scalar.dma_start(id_sb[:], ident[:, :])
            wih_sb = consts.tile([128, 4, 4 * H], BF)
            wih_r = wih.rearrange("(k p) c -> p k c", p=128)
            for blk in range(4):
                q = nc.scalar if blk % 2 == 0 else nc.sync
                q.dma_start(
                    wih_sb[:, :, blk * H : (blk + 1) * H],
                    wih_r[:, :, blk * H : (blk + 1) * H],
                )
            whh_sb = consts.tile([128, 8, 4 * H], wdtype)
            whh_r = whh.rearrange("(k p) c -> p k c", p=128)
            nc.gpsimd.dma_start(whh_sb[:, 0:4, :], whh_r[:, 0:4, :])
            nc.gpsimd.dma_start(whh_sb[:, 4:8, :], whh_r[:, 4:8, :])
            wo_sb = consts.tile([128, 8, O], BF)
            nc.gpsimd.dma_start(wo_sb[:], wo.rearrange("(k p) c -> p k c", p=128))
            bo_sb = consts.tile([128, O // 128], F32)
            nc.gpsimd.dma_start(bo_sb[:], bo[:, :])

            # xg^T: [128, chunk m, t*8+b] bf16
            xg_sb = xgp.tile([128, NM, T * BS], BF, tag="xg", name="xg")

            # ---- phase 1: xg = W_ih^T @ xs^T (+ bias), all chunks ----
            ptags = ["g0", "i0", "f0", "o0"]
            NSUB = T * BS // 512  # 2 col-halves of 512
            for m in range(NM):
                for sub in range(NSUB):
                    ps = psp.tile(
                        [128, 512], F32, tag=ptags[m % 4], name=f"ps1_{m}"
                    )
                    for k in range(4):
                        nc.tensor.matmul(
                            ps[:],
                            wih_sb[:, k, m * 128 : (m + 1) * 128],
                            xs_sb[:, k, sub * 512 : (sub + 1) * 512],
                            start=(k == 0),
                            stop=(k == 3),
                        )
                    nc.vector.tensor_scalar(
                        xg_sb[:, m, sub * 512 : (sub + 1) * 512],
                        ps[:],
                        bias_sb[:, m : m + 1],
                        None,
                        ALU.add,
                    )

            # ---- phase 2: recurrence, fully local ----
            # Per-gate PSUM split into lo/hi hidden-chunk halves (8 banks,
            # bufs=1). The step is software-pipelined: next step's k=0..3
            # matmuls need only h_lo, which is ready before this step's
            # matmuls finish, so the PE never stalls on the EW tail.
            h_half = [
                ew.tile([128, 4, BS], BF, tag=f"h{hh}", name=f"h{hh}")
                for hh in range(2)
            ]
            for hh in range(2):
                nc.vector.memset(h_half[hh][:], 0.0)
            c_half = [None, None]

            def chunk(X, hh, j):
                return BLK[X] * 8 + hh * 4 + j

            def ew_half(pst, X, hh, t):
                """EW for gate X's half-bank; updates c_half/h_half state."""
                flat = pst[(X, hh)][:].rearrange("p j b -> p (j b)")
                sfx = f"{hh}"
                if X == "g":
                    gr = ew.tile([128, 4 * BS], F32, tag=f"gr{sfx}", name=f"gr{sfx}")
                    nc.vector.tensor_scalar_max(gr[:], flat, 0.0)
                    st[("gr", hh)] = gr
                elif X == "i":
                    sig_i = ew.tile([128, 4 * BS], F32, tag=f"si{sfx}", name=f"si{sfx}")
                    nc.scalar.activation(sig_i[:], flat, AF.Sigmoid)
                    ig = ew.tile([128, 4 * BS], F32, tag=f"ig{sfx}", name=f"ig{sfx}")
                    nc.vector.tensor_tensor(
                        ig[:], sig_i[:], st[("gr", hh)][:], ALU.mult
                    )
                    st[("ig", hh)] = ig
                elif X == "f":
                    sig_f = ew.tile([128, 4 * BS], F32, tag=f"sf{sfx}", name=f"sf{sfx}")
                    nc.scalar.activation(sig_f[:], flat, AF.Sigmoid)
                    c_new = ew.tile([128, 4, BS], F32, tag=f"c{sfx}", name=f"c{sfx}")
                    cflat = c_new[:].rearrange("p j b -> p (j b)")
                    if c_half[hh] is None:
                        # c0 = 0: c = ig
                        nc.vector.tensor_copy(cflat, st[("ig", hh)][:])
                    else:
                        fc = ew.tile([128, 4 * BS], F32, tag=f"fc{sfx}", name=f"fc{sfx}")
                        nc.vector.tensor_tensor(
                            fc[:],
                            sig_f[:],
                            c_half[hh][:].rearrange("p j b -> p (j b)"),
                            ALU.mult,
                        )
                        nc.vector.tensor_tensor(
                            cflat, fc[:], st[("ig", hh)][:], ALU.add
                        )
                    c_half[hh] = c_new
                    rc = ew.tile([128, 4 * BS], F32, tag=f"rc{sfx}", name=f"rc{sfx}")
                    nc.vector.tensor_scalar(
                        rc[:], cflat, 1.0 / WS, 0.0, ALU.mult, ALU.max
                    )
                    st[("rc", hh)] = rc
                else:  # "o"
                    sig_o = ew.tile([128, 4 * BS], F32, tag=f"so{sfx}", name=f"so{sfx}")
                    nc.scalar.activation(sig_o[:], flat, AF.Sigmoid)
                    h_new = ew.tile([128, 4, BS], BF, tag=f"h{hh}", name=f"hn{hh}")
                    nc.vector.tensor_tensor(
                        h_new[:].rearrange("p j b -> p (j b)"),
                        sig_o[:],
                        st[("rc", hh)][:],
                        ALU.mult,
                    )
                    h_half[hh] = h_new

            for t in range(t_steps):
                st = {}
                first_step = t == 0
                pst = {}
                for hh in range(2):
                    for X in ORDER:
                        pst[(X, hh)] = psp.tile(
                            [128, 4, BS], F32, tag=f"{X}{hh}", name=f"ps2_{X}{hh}"
                        )
                h_in = [h_half[0], h_half[1]]
                # lo injects
                for X in ORDER:
                    m0 = BLK[X] * 8
                    nc.tensor.matmul(
                        pst[(X, 0)][:, :, :],
                        id_sb[:],
                        xg_sb[:, m0 : m0 + 4, t * BS : (t + 1) * BS],
                        start=True,
                        stop=first_step,
                    )
                if not first_step:
                    # section 1: lo banks, k 0..3 (needs h_lo only)
                    for X in ORDER:
                        for j in range(4):
                            for k in range(4):
                                nc.tensor.matmul(
                                    pst[(X, 0)][:, j, :],
                                    whh_sb[
                                        :, k,
                                        chunk(X, 0, j) * 128 : (chunk(X, 0, j) + 1) * 128,
                                    ],
                                    h_in[0][:, k, :],
                                    start=False,
                                    stop=False,
                                )
                    # section 2: lo banks, k 4..7 (needs h_hi); close lo banks
                    for X in ORDER:
                        for j in range(4):
                            for k in range(4):
                                nc.tensor.matmul(
                                    pst[(X, 0)][:, j, :],
                                    whh_sb[
                                        :, 4 + k,
                                        chunk(X, 0, j) * 128 : (chunk(X, 0, j) + 1) * 128,
                                    ],
                                    h_in[1][:, k, :],
                                    start=False,
                                    stop=(j == 3 and k == 3),
                                )
                        ew_half(pst, X, 0, t)
                else:
                    for X in ORDER:
                        ew_half(pst, X, 0, t)
                if not first_step:
                    # section 3: hi banks, k 0..7 (each gate's inject
                    # immediately precedes its matmuls)
                    for X in ORDER:
                        m0 = BLK[X] * 8
                        nc.tensor.matmul(
                            pst[(X, 1)][:, :, :],
                            id_sb[:],
                            xg_sb[:, m0 + 4 : m0 + 8, t * BS : (t + 1) * BS],
                            start=True,
                            stop=False,
                        )
                        for j in range(4):
                            for k in range(8):
                                nc.tensor.matmul(
                                    pst[(X, 1)][:, j, :],
                                    whh_sb[
                                        :, k,
                                        chunk(X, 1, j) * 128 : (chunk(X, 1, j) + 1) * 128,
                                    ],
                                    h_in[k // 4][:, k % 4, :],
                                    start=False,
                                    stop=(j == 3 and k == 7),
                                )
                        ew_half(pst, X, 1, t)
                else:
                    for X in ORDER:
                        m0 = BLK[X] * 8
                        nc.tensor.matmul(
                            pst[(X, 1)][:, :, :],
                            id_sb[:],
                            xg_sb[:, m0 + 4 : m0 + 8, t * BS : (t + 1) * BS],
                            start=True,
                            stop=True,
                        )
                        ew_half(pst, X, 1, t)

            # ---- phase 3: out^T = W_o^T @ h (+ b_o); W_o pre-scaled x128 ----
            out_sb = ew.tile([128, 4, BS], F32, tag="osb")
            for mo in range(4):
                pso = psp.tile([128, BS], F32, tag=ptags[mo], name=f"pso{mo}")
                for k in range(8):
                    nc.tensor.matmul(
                        pso[:],
                        wo_sb[:, k, mo * 128 : (mo + 1) * 128],
                        h_half[k // 4][:, k % 4, :],
                        start=(k == 0),
                        stop=(k == 7),
                    )
                nc.vector.tensor_scalar(
                    out_sb[:, mo, :],
                    pso[:],
                    bo_sb[:, mo : mo + 1],
                    None,
                    ALU.add,
                )
            nc.sync.dma_start(
                out_t.rearrange("(m p) b -> p m b", p=128), out_sb[:]
            )

    nc.compile()
    return nc


def prep_inputs(xs, W_ih, W_hh, b, W_o, b_o):
    """Host-side sharding/layout. Returns in_maps for the 8 cores."""
    xs = np.asarray(xs, dtype=np.float32)
    W_ih = np.asarray(W_ih, dtype=np.float32)
    W_hh = np.asarray(W_hh, dtype=np.float32)
    b = np.asarray(b, dtype=np.float32)
    W_o = np.asarray(W_o, dtype=np.float32)
    b_o = np.asarray(b_o, dtype=np.float32)

    # gate blocks permuted to [i, f, o, g] (orig order i, f, g, o)
    cols = np.concatenate([np.arange(g * H, (g + 1) * H) for g in (0, 1, 3, 2)])
    wih_p = np.ascontiguousarray(W_ih[:, cols]).astype(BF16)
    whh_q = np.ascontiguousarray(W_hh[:, cols] * WS).astype(FP8)
    bias_p = np.ascontiguousarray(b[cols].reshape(NM, 128).T).astype(np.float32)
    wo_p = np.ascontiguousarray(W_o * WS).astype(BF16)
    bo_p = np.ascontiguousarray(b_o.reshape(O // 128, 128).T).astype(np.float32)
    ident = np.eye(128, dtype=BF16)

    in_maps = []
    for r in range(NCORES):
        xs_r = np.ascontiguousarray(
            xs[r * BS : (r + 1) * BS].transpose(2, 1, 0).reshape(I, T * BS)
        ).astype(BF16)
        in_maps.append(
            {
                "xs_t": xs_r,
                "wih": wih_p,
                "whh": whh_q,
                "bias": bias_p,
                "wo": wo_p,
                "bo": bo_p,
                "ident": ident,
            }
        )
    return in_maps


_NC_CACHE = {}


def _get_nc(t_steps: int = T):
    if t_steps not in _NC_CACHE:
        _NC_CACHE[t_steps] = build_program(t_steps)
    return _NC_CACHE[t_steps]


def _run(inputs, trace=False):
    nc = _get_nc(T)
    in_maps = prep_inputs(**inputs)
    last_err = None
    for attempt in range(3):
        try:
            res = bass_utils.run_bass_kernel_spmd(
                nc, in_maps, core_ids=list(range(NCORES)), trace=trace
            )
            outs = [np.asarray(r["out_t"], dtype=np.float32) for r in res.results]
            out = np.concatenate([o.T for o in outs], axis=0)  # [64, 512]
            return out, res
        except Exception as e:  # noqa: BLE001 - device-transient errors
            last_err = e
            if attempt < 2:
                import time

                time.sleep(45)
    raise last_err


def kernel(**inputs) -> np.ndarray:
    out, _ = _run(inputs, trace=False)
    return out


def run_traced(**inputs):
    return _run(inputs, trace=True)
